# revision 1
# baseline (speedup 1.0000x reference)
"""Trainium2 Bass kernel for nn_ContrastiveLoss.

Full inputs: video [16, 4096, 256] f32, audio [16, 4096, 256] f32,
mask [16, 4096] int32. Output: scalar loss, shape (1,) f32.

Sharding: data-parallel over batch B=16 -> 2 samples per core on 8 cores.
Each core computes its partial (unnormalized) loss on-device; host sums the
8 partials and divides by B.

Per-core algorithm (all on device):
  idx_s = argmax(mask[s]) via a combined key max(mask*4096 + (4095-t))
  (first-max tie-break), then indirect-DMA gather of the RAW anchor rows
  video[s, idx_s], audio[s, idx_s]; anchors are broadcast unnormalized
  across 128 partitions with a tiny selector matmul on the TensorEngine
  so the main pass can start as soon as the gather lands.
  Main streaming pass (overlapped with 1MB HWDGE DMAs):
    - row dots  dot(video[s,t], a_anc_raw): DVE scalar_tensor_tensor with
      accum_out (fused multiply+row-reduce, one op per 128x256 tile)
    - row norms ||row||^2: split between ACT Square+accum_out (96 tiles)
      and DVE bn_stats (32 tiles; count/mean/M2 converted to sumsq with a
      few batched ops) to balance the two engines.
  Epilogue: anchor norms are computed late and folded into the per-stream
  exp scale (scal4[p,i] = 1/(max(||anchor_i||,eps)*TEMP), built with
  selector matmuls); sim = dots * rsqrt-like chain; per-stream
  S = sum_t exp(sim) via ACT Exp+accum + ones-matmul partition reduce;
  neg = S - exp(pos) (the t==idx term equals exp(pos) exactly);
  partial = sum_s [ -pos_s + 0.5*ln(neg_a2v) + 0.5*ln(neg_v2a) ].
  ACT table-set loads are ordered (sqrt set -> natural-log/exp set) via
  explicit deps so only ~3 loads occur, off the critical path.
"""

import numpy as np

import bass_rust
import concourse.bass as bass


class _StageDone(Exception):
    pass

import concourse.mybir as mybir
from concourse import bacc
from concourse.bass_utils import run_bass_kernel_spmd
from concourse.tile import TileContext

B, T, D = 16, 4096, 256
NCORES = 8
SPC = B // NCORES  # samples per core
TEMP = 0.07
EPS = 1e-8
P = 128
TPS = T // P  # 32 sub-tiles of 128 rows per sample
G = 8  # sub-tiles per DMA (1 MiB transfers)
NSTREAM = 2 * SPC  # (video,audio) x samples

F32 = mybir.dt.float32
BF16 = mybir.dt.bfloat16
I32 = mybir.dt.int32
# data-tile dtype: BF16 halves DVE dot cost (2x mode) at ~1e-4 loss accuracy;
# F32 keeps ~3e-7.
DT_DATA = F32
AF = mybir.ActivationFunctionType
ALU = mybir.AluOpType
AX = mybir.AxisListType


def _build(stage="full"):
    nc = bacc.Bacc(trn_type="TRN2", debug=False)

    video = nc.dram_tensor("video", [SPC, T, D], F32, kind="ExternalInput")
    audio = nc.dram_tensor("audio", [SPC, T, D], F32, kind="ExternalInput")
    mask = nc.dram_tensor("mask", [SPC, T], I32, kind="ExternalInput")
    out = nc.dram_tensor("out", [1, 1], F32, kind="ExternalOutput")

    # Constants embedded in the NEFF.
    jj = np.arange(TPS, dtype=np.float32)
    pp = np.arange(P, dtype=np.float32)[:, None]
    revt_np = 4095.0 - (pp * TPS + jj[None, :])  # [128, 32]
    revt_np = np.tile(revt_np, (1, SPC)).astype(np.float32)  # [128, 64]
    revt_d = nc.inline_tensor(revt_np, name="revt")
    ident_d = nc.inline_tensor(np.eye(P, dtype=np.float32), name="ident")
    ones_d = nc.inline_tensor(np.ones((P, 1), dtype=np.float32), name="ones")
    srow_d = nc.inline_tensor(
        (np.arange(SPC, dtype=np.int32) * T)[:, None], name="srow"
    )
    # sel[:, s*128:(s+1)*128] is a [SPC, 128] all-ones-in-row-s selector used
    # to broadcast anchor row s across 128 partitions via PE matmul.
    sel_np = np.zeros((SPC, SPC * P), dtype=np.float32)
    for s in range(SPC):
        sel_np[s, s * P : (s + 1) * P] = 1.0
    sel_d = nc.inline_tensor(sel_np, name="sel")
    ones2_d = nc.inline_tensor(np.ones((SPC, P), dtype=np.float32), name="ones2")

    vflat = video[:].rearrange("s t d -> (s t) d")
    aflat = audio[:].rearrange("s t d -> (s t) d")
    vtiles = video[:].rearrange("s (k p) d -> s p k d", p=P)  # [2,128,32,256]
    atiles = audio[:].rearrange("s (k p) d -> s p k d", p=P)
    mtile = mask[:].rearrange("s (p j) -> p s j", p=P)  # [128, 2, 32]

    with TileContext(nc) as tc:
      try:
        with (
            tc.tile_pool(name="sb", bufs=1) as sb,
            tc.tile_pool(name="data", bufs=6) as data,
            tc.tile_pool(name="scr", bufs=2) as scr,
            tc.tile_pool(name="ps", bufs=2, space="PSUM") as ps,
            tc.tile_pool(name="ps1", bufs=1, space="PSUM") as ps1,
        ):
            def st(shape, tag, dtype=F32, pool=sb):
                return pool.tile(shape, dtype, tag=tag, name=tag)

            # ---- constants to SBUF ----
            revt = st([P, SPC * TPS], "revt")
            ident = st([P, P], "ident")
            ones = st([P, 1], "ones")
            srow = st([SPC, 1], "srow", dtype=I32)
            sel = st([SPC, SPC * P], "sel")
            ones2 = st([SPC, P], "ones2")
            nc.sync.dma_start(out=revt[:], in_=revt_d[:])
            nc.sync.dma_start(out=ident[:], in_=ident_d[:])
            nc.sync.dma_start(out=ones[:], in_=ones_d[:])
            nc.sync.dma_start(out=srow[:], in_=srow_d[:])
            nc.sync.dma_start(out=sel[:], in_=sel_d[:])
            nc.sync.dma_start(out=ones2[:], in_=ones2_d[:])

            # stream-0 groups 0/1 early + fine-grained so the first tiles
            # land fast (subtile deps let compute start per 256KB chunk)
            bigs0 = []
            for g in range(TPS // G):
                bigs0.append(data.tile([P, G, D], DT_DATA, tag="big", name="big"))
            _eng = nc.sync if DT_DATA is F32 else nc.gpsimd
            for g, nsub in ((0, 4), (1, 2)):
                step = G // nsub
                for j in range(nsub):
                    _eng.dma_start(
                        out=bigs0[g][:, j * step : (j + 1) * step, :],
                        in_=vtiles[0, :, g * G + j * step : g * G + (j + 1) * step, :],
                    )

            warm = st([1, 1], "warm")
            nc.scalar.activation(out=warm[:], in_=ones[0:1, :], func=AF.Sqrt)

            # ---- Stage A: argmax of mask per sample ----
            mf = st([P, SPC * TPS], "mf")
            mf3 = mf[:].rearrange("p (s j) -> p s j", s=SPC)
            nc.gpsimd.dma_start(out=mf3, in_=mtile)  # int32 -> f32 cast
            comb = st([P, SPC * TPS], "comb")
            nc.vector.scalar_tensor_tensor(
                out=comb[:], in0=mf[:], scalar=4096.0, in1=revt[:],
                op0=ALU.mult, op1=ALU.add,
            )
            pmax = st([P, SPC], "pmax")
            for s in range(SPC):
                nc.vector.reduce_max(
                    out=pmax[:, s : s + 1],
                    in_=comb[:, s * TPS : (s + 1) * TPS],
                    axis=AX.X,
                )
            pmax_t = ps1.tile([SPC, P], F32, tag="pmax_t", name="pmax_t")
            nc.tensor.transpose(out=pmax_t[:], in_=pmax[:], identity=ident[:])
            Mf = st([SPC, 1], "Mf")
            nc.vector.reduce_max(out=Mf[:], in_=pmax_t[:], axis=AX.X)
            Mi = st([SPC, 1], "Mi", dtype=I32)
            nc.vector.tensor_copy(out=Mi[:], in_=Mf[:])
            # idx = 4095 - (M & 4095); gidx = idx + s*T
            ridx = st([SPC, 1], "ridx", dtype=I32)
            nc.vector.tensor_scalar(
                out=ridx[:], in0=Mi[:], scalar1=4095, scalar2=None,
                op0=ALU.bitwise_and,
            )
            idx = st([SPC, 1], "idx", dtype=I32)
            nc.vector.tensor_scalar(
                out=idx[:], in0=ridx[:], scalar1=-1, scalar2=4095,
                op0=ALU.mult, op1=ALU.add,
            )
            gidx = st([SPC, 1], "gidx", dtype=I32)
            nc.vector.tensor_tensor(
                out=gidx[:], in0=idx[:], in1=srow[:], op=ALU.add
            )

            if stage == "A":
                dbg = st([1, 1], "dbg")
                nc.vector.tensor_copy(out=dbg[:], in_=gidx[0:1, :])
                nc.sync.dma_start(out=out[:], in_=dbg[:])
            if stage == "A":
                raise _StageDone(nc)

            # ---- Stage B: gather + normalize anchors, pos ----
            anc_v = st([SPC, D], "anc_v")
            anc_a = st([SPC, D], "anc_a")
            nc.gpsimd.indirect_dma_start(
                out=anc_v[:], out_offset=None, in_=vflat,
                in_offset=bass.IndirectOffsetOnAxis(ap=gidx[:, :1], axis=0),
            )
            nc.gpsimd.indirect_dma_start(
                out=anc_a[:], out_offset=None, in_=aflat,
                in_offset=bass.IndirectOffsetOnAxis(ap=gidx[:, :1], axis=0),
            )
            if stage == "B1":
                nc.sync.dma_start(out=out[:], in_=anc_v[0:1, 0:1])
                raise _StageDone(nc)
            dots = st([P, NSTREAM * TPS], "dots")
            ssq = st([P, NSTREAM * TPS], "ssq")
            # per-stream count of sumsq tiles handled by DVE bn_stats
            DCNT = [16, 16, 0, 0]
            DOFF = [0]
            for _d in DCNT[:-1]:
                DOFF.append(DOFF[-1] + _d)
            NDVE = sum(DCNT)
            bnraw = st([P, NDVE * 6], "bnraw")

            plans = []
            for i in range(NSTREAM):
                dc = DCNT[i]
                if dc:
                    dve_ks = [k for k in range(TPS) if k % 2 == 0][:dc]
                else:
                    dve_ks = []
                dve_rank = {k: r for r, k in enumerate(dve_ks)}
                act_ks = [k for k in range(TPS) if k not in dve_rank]
                colof = {}
                for r, k in enumerate(act_ks):
                    colof[k] = i * TPS + r
                for r, k in enumerate(dve_ks):
                    colof[k] = i * TPS + (TPS - dc) + r
                plans.append((dc, dve_rank, colof))

            sq_insts = []

            def emit_sumsq(i, big, kk, k):
                dc, dve_rank, colof = plans[i]
                col = colof[k]
                if k in dve_rank:
                    slot = DOFF[i] + dve_rank[k]
                    nc.vector.bn_stats(
                        out=bnraw[:, slot * 6 : slot * 6 + 6],
                        in_=big[:, kk, :],
                    )
                else:
                    psc = ps.tile([P, D], F32, tag="psc", name="psc")
                    sq_insts.append(nc.scalar.activation(
                        out=psc[:], in_=big[:, kk, :], func=AF.Square,
                        accum_out=ssq[:, col : col + 1],
                    ))

            def emit_dot(i, big, kk, k):
                dc, dve_rank, colof = plans[i]
                col = colof[k]
                dscr = scr.tile([P, D], DT_DATA, tag="dscr", name="dscr")
                nc.vector.scalar_tensor_tensor(
                    out=dscr[:], in0=big[:, kk, :], scalar=1.0,
                    in1=bcast[i][:], op0=ALU.mult, op1=ALU.mult,
                    accum_out=dots[:, col : col + 1],
                )

            def emit_conv(i):
                # bn stats -> ssq columns:
                # sumsq = (M2_0 + 128*m0^2) + (M2_1 + 128*m1^2)
                dc, dve_rank, colof = plans[i]
                if dc == 0:
                    return
                raw = bnraw[:].rearrange("p (n six) -> p n six", six=6)
                sl = slice(DOFF[i], DOFF[i] + dc)
                m0, m1 = raw[:, sl, 1], raw[:, sl, 4]
                M20, M21 = raw[:, sl, 2], raw[:, sl, 5]
                c0 = i * TPS + (TPS - dc)
                tgt = ssq[:, c0 : c0 + dc]
                cva = scr.tile([P, 32], F32, tag="cva", name="cva")
                cvb = scr.tile([P, 32], F32, tag="cvb", name="cvb")
                nc.vector.tensor_tensor(
                    out=cva[:, :dc], in0=m0, in1=m0, op=ALU.mult)
                nc.vector.tensor_tensor(
                    out=cvb[:, :dc], in0=m1, in1=m1, op=ALU.mult)
                nc.vector.tensor_tensor(
                    out=cva[:, :dc], in0=cva[:, :dc], in1=cvb[:, :dc],
                    op=ALU.add)
                nc.vector.scalar_tensor_tensor(
                    out=cvb[:, :dc], in0=cva[:, :dc], scalar=float(P),
                    in1=M20, op0=ALU.mult, op1=ALU.add)
                nc.vector.tensor_tensor(
                    out=tgt, in0=cvb[:, :dc], in1=M21, op=ALU.add)

            def emit_dma(tiles_ap, s, g):
                big = data.tile([P, G, D], DT_DATA, tag="big", name="big")
                eng = nc.sync if DT_DATA is F32 else nc.gpsimd
                eng.dma_start(
                    out=big[:], in_=tiles_ap[s, :, g * G : (g + 1) * G, :])
                return big

            # Phase A: stream 0 sumsq-only (keeps DVE+ACT busy while the
            # anchor gather/broadcast chain resolves); dots deferred.
            for g in (0, 1):
                for kk in range(G):
                    emit_sumsq(0, bigs0[g], kk, g * G + kk)

            # ---- Stage C: broadcast RAW anchors across partitions ----
            # (anchor normalization is deferred to the epilogue and folded
            # into the per-stream exp scale.)
            # video rows pair with the audio anchor; audio rows with video's.
            bcast = []  # stream i = s*2 + (0 video, 1 audio)
            for s in range(SPC):
                bv = st([P, D], f"bv{s}", dtype=DT_DATA)
                ba = st([P, D], f"ba{s}", dtype=DT_DATA)
                bv_ps = ps.tile([P, D], F32, tag="bc_ps", name="bv_ps")
                ba_ps = ps.tile([P, D], F32, tag="bc_ps", name="ba_ps")
                nc.tensor.matmul(
                    out=bv_ps[:], lhsT=sel[:, s * P : (s + 1) * P],
                    rhs=anc_a[:], start=True, stop=True,
                )
                nc.tensor.matmul(
                    out=ba_ps[:], lhsT=sel[:, s * P : (s + 1) * P],
                    rhs=anc_v[:], start=True, stop=True,
                )
                nc.vector.tensor_copy(out=bv[:], in_=bv_ps[:])
                nc.vector.tensor_copy(out=ba[:], in_=ba_ps[:])
                bcast += [bv, ba]

            if stage == "C":
                dbg2 = st([1, 1], "dbg2")
                nc.vector.reduce_sum(out=dbg2[:], in_=bcast[3][0:1, :], axis=AX.X)
                nc.sync.dma_start(out=out[:], in_=dbg2[:])
                raise _StageDone(nc)

            for g in (2, 3):
                nc.sync.dma_start(
                    out=bigs0[g][:],
                    in_=vtiles[0, :, g * G : (g + 1) * G, :],
                ) if DT_DATA is F32 else nc.gpsimd.dma_start(
                    out=bigs0[g][:],
                    in_=vtiles[0, :, g * G : (g + 1) * G, :],
                )
                for kk in range(G):
                    emit_sumsq(0, bigs0[g], kk, g * G + kk)
            emit_conv(0)

            # ---- Stage D phase B: stream-0 dots, then streams 1..3 ----
            for g in range(TPS // G):
                for kk in range(G):
                    emit_dot(0, bigs0[g], kk, g * G + kk)
            for i in range(1, NSTREAM):
                s, si = divmod(i, 2)
                tiles_ap = vtiles if si == 0 else atiles
                for g in range(TPS // G):
                    if i == 1:
                        big = data.tile([P, G, D], DT_DATA, tag="big", name="big")
                        eng = nc.sync if DT_DATA is F32 else nc.gpsimd
                        h = G // 2
                        for j in range(2):
                            eng.dma_start(
                                out=big[:, j * h : (j + 1) * h, :],
                                in_=tiles_ap[s, :, g * G + j * h : g * G + (j + 1) * h, :],
                            )
                    else:
                        big = emit_dma(tiles_ap, s, g)
                    for kk in range(G):
                        k = g * G + kk
                        emit_dot(i, big, kk, k)
                        emit_sumsq(i, big, kk, k)
                emit_conv(i)

            # force ACT queue order: anchor-norm sqrts after most squares
            mid_sq = sq_insts[min(60, len(sq_insts) - 1)]

            if stage == "D":
                dbg3 = st([1, 1], "dbg3")
                nc.vector.reduce_sum(out=dbg3[:], in_=dots[0:1, :], axis=AX.X)
                nc.sync.dma_start(out=out[:], in_=dbg3[:])
                raise _StageDone(nc)

            # ---- Stage E: epilogue ----
            # deferred anchor normalization (off the main-pass critical path)
            scr2 = st([SPC, D], "scr2")
            scr3 = st([SPC, D], "scr3")
            ssv = st([SPC, 1], "ssv")
            ssa = st([SPC, 1], "ssa")
            nc.vector.scalar_tensor_tensor(
                out=scr2[:], in0=anc_v[:], scalar=1.0, in1=anc_v[:],
                op0=ALU.mult, op1=ALU.mult, accum_out=ssv[:],
            )
            nc.vector.scalar_tensor_tensor(
                out=scr3[:], in0=anc_a[:], scalar=1.0, in1=anc_a[:],
                op0=ALU.mult, op1=ALU.mult, accum_out=ssa[:],
            )
            nv = st([SPC, 1], "nv")
            na = st([SPC, 1], "na")
            nv_i = nc.scalar.activation(out=nv[:], in_=ssv[:], func=AF.Sqrt)
            na_i = nc.scalar.activation(out=na[:], in_=ssa[:], func=AF.Sqrt)
            bass_rust.add_dep_helper(
                nv_i.ins, mid_sq.ins, sync=True,
                reason="ACT queue: anchor sqrts after main squares")
            bass_rust.add_dep_helper(
                na_i.ins, mid_sq.ins, sync=True,
                reason="ACT queue: anchor sqrts after main squares")
            nc.vector.tensor_scalar_max(out=nv[:], in0=nv[:], scalar1=EPS)
            nc.vector.tensor_scalar_max(out=na[:], in0=na[:], scalar1=EPS)
            rv = st([SPC, 1], "rv")
            ra = st([SPC, 1], "ra")
            nc.vector.reciprocal(out=rv[:], in_=nv[:])
            nc.vector.reciprocal(out=ra[:], in_=na[:])
            # scal4[p, 2q+j] = (inv-norm of the anchor used by stream 2q+j)
            # / TEMP, identical on every partition. rvec = [ra|rv]/TEMP; the
            # sel-matmul per sample q broadcasts row q across partitions.
            rvec = st([SPC, 2], "rvec")
            nc.vector.tensor_scalar(
                out=rvec[:, 0:1], in0=ra[:], scalar1=1.0 / TEMP, scalar2=None,
                op0=ALU.mult)
            nc.vector.tensor_scalar(
                out=rvec[:, 1:2], in0=rv[:], scalar1=1.0 / TEMP, scalar2=None,
                op0=ALU.mult)
            scal4_ps = ps1.tile([P, NSTREAM], F32, tag="scal4_ps", name="scal4_ps")
            for q in range(SPC):
                nc.tensor.matmul(
                    out=scal4_ps[:, 2 * q : 2 * q + 2],
                    lhsT=sel[:, q * P : (q + 1) * P], rhs=rvec[:],
                    start=True, stop=True)
            scal4 = st([P, NSTREAM], "scal4")
            nc.vector.tensor_copy(out=scal4[:], in_=scal4_ps[:])
            # pos = <anc_v, anc_a> * rv * ra / TEMP
            praw = st([SPC, 1], "praw")
            scr4 = st([SPC, D], "scr4")
            nc.vector.scalar_tensor_tensor(
                out=scr4[:], in0=anc_v[:], scalar=1.0, in1=anc_a[:],
                op0=ALU.mult, op1=ALU.mult, accum_out=praw[:],
            )
            q1 = st([SPC, 1], "q1")
            nc.vector.tensor_tensor(out=q1[:], in0=rv[:], in1=ra[:], op=ALU.mult)
            pos = st([SPC, 1], "pos")
            nc.vector.scalar_tensor_tensor(
                out=pos[:], in0=praw[:], scalar=1.0 / TEMP, in1=q1[:],
                op0=ALU.mult, op1=ALU.mult,
            )

            W = NSTREAM * TPS  # 128
            nall = st([P, W], "nall")
            rall = st([P, W], "rall")
            sim = st([P, W], "sim")
            sqrt_inst = nc.scalar.activation(
                out=nall[:], in_=ssq[:], func=AF.Sqrt)
            nc.vector.tensor_scalar_max(
                out=nall[:], in0=nall[:], scalar1=EPS)
            nc.vector.reciprocal(out=rall[:], in_=nall[:])
            nc.vector.tensor_tensor(
                out=sim[:], in0=dots[:], in1=rall[:], op=ALU.mult)
            # preload the exp/ln table set while DVE finishes the last chain
            ewarm = st([1, 1], "ewarm")
            ewarm_i = nc.scalar.activation(
                out=ewarm[:], in_=warm[:], func=AF.Ln)
            bass_rust.add_dep_helper(
                ewarm_i.ins, sqrt_inst.ins, sync=True,
                reason="exp/ln table load after last stats sqrt")
            eall = st([P, W], "eall")
            Scols = st([P, NSTREAM], "Scols")
            for i in range(NSTREAM):
                e_i = nc.scalar.activation(
                    out=eall[:, i * TPS : (i + 1) * TPS],
                    in_=sim[:, i * TPS : (i + 1) * TPS],
                    func=AF.Exp, scale=scal4[:, i : i + 1],
                    accum_out=Scols[:, i : i + 1],
                )
                bass_rust.add_dep_helper(
                    e_i.ins, sqrt_inst.ins, sync=True,
                    reason="all exps after last sqrt (one table switch)")
            SP = ps1.tile([1, NSTREAM], F32, tag="SP", name="SP")
            nc.tensor.matmul(
                out=SP[:], lhsT=ones[:], rhs=Scols[:], start=True, stop=True
            )
            pos_t = ps1.tile([1, SPC], F32, tag="pos_t", name="pos_t")
            nc.tensor.transpose(
                out=pos_t[:], in_=pos[:], identity=ident[:SPC, :SPC]
            )
            # epi layout: [0:4] S per stream, [4:6] pos, [6:8] exp(pos)
            epi = st([1, 8], "epi")
            nc.vector.tensor_copy(out=epi[:, 0:NSTREAM], in_=SP[:])
            nc.vector.tensor_copy(out=epi[:, 4 : 4 + SPC], in_=pos_t[:, 0:SPC])
            pexp_inst = nc.scalar.activation(
                out=epi[:, 6 : 6 + SPC], in_=epi[:, 4 : 4 + SPC], func=AF.Exp
            )
            # keep the exp/ln table load after the sqrt-set uses
            bass_rust.add_dep_helper(
                pexp_inst.ins, sqrt_inst.ins, sync=True,
                reason="group ACT table sets: exp after sqrt")
            negs = st([1, NSTREAM], "negs")
            for s in range(SPC):
                nc.vector.tensor_scalar(
                    out=negs[:, 2 * s : 2 * s + 2],
                    in0=epi[:, 2 * s : 2 * s + 2],
                    scalar1=epi[:, 6 + s : 7 + s],
                    scalar2=None,
                    op0=ALU.subtract,
                )
            lns = st([1, NSTREAM], "lns")
            nc.scalar.activation(out=lns[:], in_=negs[:], func=AF.Ln)
            lsum = st([1, 1], "lsum")
            nc.vector.reduce_sum(out=lsum[:], in_=lns[:], axis=AX.X)
            psum2 = st([1, 1], "psum2")
            nc.vector.reduce_sum(
                out=psum2[:], in_=epi[:, 4 : 4 + SPC], axis=AX.X
            )
            res = st([1, 1], "res")
            nc.vector.scalar_tensor_tensor(
                out=res[:], in0=lsum[:], scalar=0.5, in1=psum2[:],
                op0=ALU.mult, op1=ALU.subtract,
            )
            nc.sync.dma_start(out=out[:], in_=res[:])
      except _StageDone:
        pass

    nc.compile()
    return nc


_NC = None


def _get_nc():
    global _NC
    if _NC is None:
        _NC = _build()
    return _NC


def kernel(video, audio, mask):
    video = np.asarray(video, dtype=np.float32)
    audio = np.asarray(audio, dtype=np.float32)
    mask = np.asarray(mask, dtype=np.int32)
    nc = _get_nc()
    in_maps = []
    for c in range(NCORES):
        sl = slice(c * SPC, (c + 1) * SPC)
        in_maps.append(
            {
                "video": np.ascontiguousarray(video[sl]),
                "audio": np.ascontiguousarray(audio[sl]),
                "mask": np.ascontiguousarray(mask[sl]),
            }
        )
    res = run_bass_kernel_spmd(nc, in_maps, core_ids=list(range(NCORES)))
    total = np.float64(0.0)
    for r in res.results:
        total += np.float64(r["out"][0, 0])
    return np.array([total / B], dtype=np.float32)



# revision 12
# speedup vs baseline: 1.1549x; 1.1549x over previous
"""Trainium2 Bass kernel for nn_ContrastiveLoss (v3: transposed layout).

Full inputs: video [16, 4096, 256] f32, audio [16, 4096, 256] f32,
mask [16, 4096] int32. Output: scalar loss, shape (1,) f32.

Sharding: data-parallel over batch B=16 -> 2 samples per core on 8 cores.
Host sums the 8 partial losses and divides by B.

Per-core plan (all heavy data in bf16, TRANSPOSED host-side to [s, d, t]):
  - mask argmax (first-max tie-break) -> idx_s; indirect-DMA gather of the
    f32 anchor rows from a natural-layout f32 copy (read-only, 2 rows).
  - anchors transposed on PE to per-partition columns, cast to bf16; they
    become matmul WEIGHTS.
  - squares: sq = x*x elementwise on ACT (Square) / DVE (tt 2x) / GPSIMD,
    one [128, 2048] half-tile at a time, bf16.
  - PE reduces over d (the partition dim): dot rows via lhsT=anchor column,
    sumsq rows via lhsT=ones; [8, 512]-chunk PSUM tiles accumulate the two
    128-d chunks; a single DVE/ACT copy evacuates 4096 values at once.
  - results bounce through a DRAM scratch to land t-on-partitions
    ([128, 128] f32 natural tiles), then the scalar epilogue (same as the
    f32 baseline): sim = dots * rsqrt(ssq) * 1/(||anc||*TEMP); per-stream
    S = sum_t exp(sim); neg = S - exp(pos); loss partial from pos/negs.
"""

import numpy as np

import bass_rust
import concourse.bass as bass
import concourse.mybir as mybir
from concourse import bacc
from concourse.bass_utils import run_bass_kernel_spmd
from concourse.tile import TileContext

B, T, D = 16, 4096, 256
NCORES = 8
SPC = B // NCORES          # samples per core
TEMP = 0.07
EPS = 1e-8
P = 128
TPS = T // P               # 32 (natural epilogue tiling: t = p*TPS + j)
NST = 2 * SPC              # streams: (s, video/audio)
NCH = D // P               # 2 d-chunks
JC = 512                   # PSUM reduce chunk (t columns)
NJ = T // JC               # 8 j-chunks per stream
HALF = T // 2              # square half-tile

F32 = mybir.dt.float32
BF16 = mybir.dt.bfloat16
I32 = mybir.dt.int32
AF = mybir.ActivationFunctionType
ALU = mybir.AluOpType
AX = mybir.AxisListType

# static engine assignment for the 16 squaring half-tiles, keyed by
# (stream, chunk, half) order of arrival; balanced so DVE/ACT/GPSIMD all
# finish early and the last halves go to the fastest engines.
# streams arrive v0, a0, v1, a1; chunks c0, c1; halves h0, h1.
_SQ_ENGINE = {}
_order = [(q, c, h) for q in range(NST) for c in range(NCH) for h in range(2)]
_pattern = ["ACT", "DVE", "ACT", "GP", "ACT", "DVE", "ACT", "GP",
            "ACT", "DVE", "ACT", "DVE", "ACT", "DVE", "DVE", "DVE"]
for _k, _e in zip(_order, _pattern):
    _SQ_ENGINE[_k] = _e


def _build():
    nc = bacc.Bacc(trn_type="TRN2", debug=False)

    vT = nc.dram_tensor("vT", [SPC, D, T], BF16, kind="ExternalInput")
    aT = nc.dram_tensor("aT", [SPC, D, T], BF16, kind="ExternalInput")
    v32 = nc.dram_tensor("v32", [SPC, T, D], F32, kind="ExternalInput")
    a32 = nc.dram_tensor("a32", [SPC, T, D], F32, kind="ExternalInput")
    mask = nc.dram_tensor("mask", [SPC, T], I32, kind="ExternalInput")
    scr = nc.dram_tensor("scr", [2, NST, T], F32, kind="Internal")
    out = nc.dram_tensor("out", [1, 1], F32, kind="ExternalOutput")

    # constants
    jj = np.arange(TPS, dtype=np.float32)
    pp = np.arange(P, dtype=np.float32)[:, None]
    revt_np = 4095.0 - (pp * TPS + jj[None, :])         # [128, 32]
    revt_np = np.tile(revt_np, (1, SPC)).astype(np.float32)
    revt_d = nc.inline_tensor(revt_np, name="revt")
    ident_d = nc.inline_tensor(np.eye(P, dtype=np.float32), name="ident")
    ones_d = nc.inline_tensor(np.ones((P, 1), dtype=np.float32), name="ones")
    srow_d = nc.inline_tensor(
        (np.arange(SPC, dtype=np.int32) * T)[:, None], name="srow")
    sel_np = np.zeros((SPC, SPC * P), dtype=np.float32)
    for s in range(SPC):
        sel_np[s, s * P : (s + 1) * P] = 1.0
    sel_d = nc.inline_tensor(sel_np, name="sel")

    vflat = v32[:].rearrange("s t d -> (s t) d")
    aflat = a32[:].rearrange("s t d -> (s t) d")
    mtile = mask[:].rearrange("s (p j) -> p s j", p=P)   # [128, SPC, 32]
    scr4 = scr[:]                                        # [2, NST, T]

    with TileContext(nc) as tc:
        with (
            tc.tile_pool(name="sb", bufs=1) as sb,
            tc.tile_pool(name="data", bufs=6) as data,
            tc.tile_pool(name="sqp", bufs=5) as sqp,
            tc.tile_pool(name="ev", bufs=3) as evp,
            tc.tile_pool(name="ps", bufs=3, space="PSUM") as ps,
            tc.tile_pool(name="ps1", bufs=1, space="PSUM") as ps1,
        ):
            def st(shape, tag, dtype=F32, pool=sb):
                return pool.tile(shape, dtype, tag=tag, name=tag)

            # ---- constants to SBUF ----
            revt = st([P, SPC * TPS], "revt")
            ident = st([P, P], "ident")
            ones = st([P, 1], "ones")
            srow = st([SPC, 1], "srow", dtype=I32)
            sel = st([SPC, SPC * P], "sel")
            nc.sync.dma_start(out=revt[:], in_=revt_d[:])
            nc.sync.dma_start(out=ident[:], in_=ident_d[:])
            nc.sync.dma_start(out=ones[:], in_=ones_d[:])
            nc.sync.dma_start(out=srow[:], in_=srow_d[:])
            nc.sync.dma_start(out=sel[:], in_=sel_d[:])
            ones16 = st([P, 1], "ones16", dtype=BF16)
            nc.vector.tensor_copy(out=ones16[:], in_=ones[:])

            # ---- bulk loads: per (stream, chunk) [128, T] bf16, 2 halves
            bigs = {}
            srcT = {0: vT, 1: aT}
            for q in range(NST):
                s, tn = divmod(q, 2)
                for c in range(NCH):
                    big = data.tile([P, T], BF16, tag="big", name="big")
                    for h in range(2):
                        nc.sync.dma_start(
                            out=big[:, h * HALF : (h + 1) * HALF],
                            in_=srcT[tn][s, c * P : (c + 1) * P,
                                         h * HALF : (h + 1) * HALF],
                        )
                    bigs[(q, c)] = big

            # warm the ACT sqrt/square table set off the critical path
            warm = st([1, 1], "warm")
            nc.scalar.activation(out=warm[:], in_=ones[0:1, :], func=AF.Sqrt)

            # ---- Stage A: argmax of mask per sample ----
            mf = st([P, SPC * TPS], "mf")
            mf3 = mf[:].rearrange("p (s j) -> p s j", s=SPC)
            nc.gpsimd.dma_start(out=mf3, in_=mtile)  # int32 -> f32 cast
            comb = st([P, SPC * TPS], "comb")
            nc.vector.scalar_tensor_tensor(
                out=comb[:], in0=mf[:], scalar=4096.0, in1=revt[:],
                op0=ALU.mult, op1=ALU.add,
            )
            pmax = st([P, SPC], "pmax")
            for s in range(SPC):
                nc.vector.reduce_max(
                    out=pmax[:, s : s + 1],
                    in_=comb[:, s * TPS : (s + 1) * TPS],
                    axis=AX.X,
                )
            pmax_t = ps1.tile([SPC, P], F32, tag="pmax_t", name="pmax_t")
            nc.tensor.transpose(out=pmax_t[:], in_=pmax[:], identity=ident[:])
            Mf = st([SPC, 1], "Mf")
            nc.vector.reduce_max(out=Mf[:], in_=pmax_t[:], axis=AX.X)
            Mi = st([SPC, 1], "Mi", dtype=I32)
            nc.vector.tensor_copy(out=Mi[:], in_=Mf[:])
            ridx = st([SPC, 1], "ridx", dtype=I32)
            nc.vector.tensor_scalar(
                out=ridx[:], in0=Mi[:], scalar1=4095, scalar2=None,
                op0=ALU.bitwise_and,
            )
            idx = st([SPC, 1], "idx", dtype=I32)
            nc.vector.tensor_scalar(
                out=idx[:], in0=ridx[:], scalar1=-1, scalar2=4095,
                op0=ALU.mult, op1=ALU.add,
            )
            gidx = st([SPC, 1], "gidx", dtype=I32)
            nc.vector.tensor_tensor(
                out=gidx[:], in0=idx[:], in1=srow[:], op=ALU.add)

            # ---- Stage B: gather f32 anchors ----
            anc_v = st([SPC, D], "anc_v")
            anc_a = st([SPC, D], "anc_a")
            nc.gpsimd.indirect_dma_start(
                out=anc_v[:], out_offset=None, in_=vflat,
                in_offset=bass.IndirectOffsetOnAxis(ap=gidx[:, :1], axis=0),
            )
            nc.gpsimd.indirect_dma_start(
                out=anc_a[:], out_offset=None, in_=aflat,
                in_offset=bass.IndirectOffsetOnAxis(ap=gidx[:, :1], axis=0),
            )

            # ---- Stage C: transpose anchors to per-partition columns ----
            # ancT16 cols: c * NST + tensor * SPC + s  (tensor 0=v, 1=a)
            ancT16 = st([P, NCH * NST], "ancT16", dtype=BF16)
            for c in range(NCH):
                for tn, anc in ((0, anc_v), (1, anc_a)):
                    atp = ps1.tile([P, SPC], F32, tag="atp", name="atp")
                    nc.tensor.transpose(
                        out=atp[:], in_=anc[:, c * P : (c + 1) * P],
                        identity=ident[:SPC, :SPC],
                    )
                    o = c * NST + tn * SPC
                    nc.vector.tensor_copy(
                        out=ancT16[:, o : o + SPC], in_=atp[:])

            # dot-weight column for stream q chunk c: the OTHER tensor's
            # anchor of the same sample.
            def dotw(q, c):
                s, tn = divmod(q, 2)
                col = c * NST + (1 - tn) * SPC + s
                return ancT16[:, col : col + 1]

            # PE out base partition must be 0: each reduce matmul writes the
            # full [8, JC] tile through a [128, 8] weight mat that is zero
            # except its target column. Static ones-mats for sq rows
            # (col 2jj+1), runtime anchor-mats for dot rows (col 2jj).
            onesm_np = np.zeros((P, 4 * 8), dtype=np.float32)
            for _jj in range(4):
                onesm_np[:, _jj * 8 + 2 * _jj + 1] = 1.0
            onesm_d = nc.inline_tensor(onesm_np, name="onesm")
            onesm_f = st([P, 32], "onesm_f")
            nc.sync.dma_start(out=onesm_f[:], in_=onesm_d[:])
            onesm = st([P, 32], "onesm", dtype=BF16)
            nc.vector.tensor_copy(out=onesm[:], in_=onesm_f[:])

            wqs = {}
            for q in range(NST):
                for c in range(NCH):
                    wq = st([P, 32], f"wq{q}{c}", dtype=BF16)
                    nc.gpsimd.memset(wq[:], 0.0)
                    for jj in range(4):
                        nc.vector.tensor_copy(
                            out=wq[:, jj * 8 + 2 * jj : jj * 8 + 2 * jj + 1],
                            in_=dotw(q, c),
                        )
                    wqs[(q, c)] = wq

            # ---- Stage D: squares + PE reduces + evac/bounce ----
            sq_insts = []
            sqs = {}
            for q in range(NST):
                for c in range(NCH):
                    big = bigs[(q, c)]
                    sq = sqp.tile([P, T], BF16, tag="sq", name="sq")
                    for h in range(2):
                        eng = _SQ_ENGINE[(q, c, h)]
                        sl = slice(h * HALF, (h + 1) * HALF)
                        if eng == "ACT":
                            i = nc.scalar.activation(
                                out=sq[:, sl], in_=big[:, sl], func=AF.Square)
                            sq_insts.append(i)
                        elif eng == "DVE":
                            nc.vector.tensor_tensor(
                                out=sq[:, sl], in0=big[:, sl], in1=big[:, sl],
                                op=ALU.mult)
                        else:
                            nc.gpsimd.tensor_tensor(
                                out=sq[:, sl], in0=big[:, sl], in1=big[:, sl],
                                op=ALU.mult)
                    sqs[(q, c)] = sq

            # PE reduce per stream, j-group of 4: PSUM [8, 512]
            # rows: (j_in_group * 2) + {0: dot, 1: sq}
            evac_dmas = []
            for q in range(NST):
                for g in range(2):
                    pt = ps.tile([8, JC], F32, tag="red", name="red")
                    nmm = 4 * NCH * 2
                    k = 0
                    for jj in range(4):
                        j = g * 4 + jj
                        sl = slice(j * JC, (j + 1) * JC)
                        for c in range(NCH):
                            nc.tensor.matmul(
                                out=pt[:],
                                lhsT=wqs[(q, c)][:, jj * 8 : (jj + 1) * 8],
                                rhs=bigs[(q, c)][:, sl],
                                start=(k == 0), stop=(k == nmm - 1),
                            )
                            k += 1
                            nc.tensor.matmul(
                                out=pt[:],
                                lhsT=onesm[:, jj * 8 : (jj + 1) * 8],
                                rhs=sqs[(q, c)][:, sl],
                                start=(k == 0), stop=(k == nmm - 1),
                            )
                            k += 1
                    ev = evp.tile([8, JC], F32, tag="ev", name="ev")
                    if q % 2 == 0:
                        nc.vector.tensor_copy(out=ev[:], in_=pt[:])
                    else:
                        nc.scalar.activation(
                            out=ev[:], in_=pt[:], func=AF.Copy)
                    # rows (j, qty) -> scr[qty, q, j*JC : ...]
                    ed = nc.sync.dma_start(
                        out=scr4[:, q, g * 4 * JC : (g + 1) * 4 * JC]
                        .rearrange("qty (j cc) -> j qty cc", cc=JC),
                        in_=ev[:],
                    )
                    evac_dmas.append(ed)

            # ---- reload natural [128, NST*TPS] ----
            dots = st([P, NST * TPS], "dots")
            ssq = st([P, NST * TPS], "ssq")
            # col = q*TPS + j ; t = p*TPS + j
            rl1 = nc.sync.dma_start(
                out=dots[:].rearrange("p (q j) -> p q j", j=TPS),
                in_=scr4[0].rearrange("q (p j) -> p q j", j=TPS),
            )
            rl2 = nc.sync.dma_start(
                out=ssq[:].rearrange("p (q j) -> p q j", j=TPS),
                in_=scr4[1].rearrange("q (p j) -> p q j", j=TPS),
            )
            # Tile does not track DRAM read-after-write: order the natural
            # reloads after every scratch evacuation explicitly.
            for ed in evac_dmas:
                bass_rust.add_dep_helper(
                    rl1.ins, ed.ins, sync=True, reason="scr RAW")
                bass_rust.add_dep_helper(
                    rl2.ins, ed.ins, sync=True, reason="scr RAW")

            # ---- Stage E: epilogue (same math as baseline) ----
            scr2 = st([SPC, D], "scr2")
            scr3 = st([SPC, D], "scr3")
            ssv = st([SPC, 1], "ssv")
            ssa = st([SPC, 1], "ssa")
            nc.vector.scalar_tensor_tensor(
                out=scr2[:], in0=anc_v[:], scalar=1.0, in1=anc_v[:],
                op0=ALU.mult, op1=ALU.mult, accum_out=ssv[:],
            )
            nc.vector.scalar_tensor_tensor(
                out=scr3[:], in0=anc_a[:], scalar=1.0, in1=anc_a[:],
                op0=ALU.mult, op1=ALU.mult, accum_out=ssa[:],
            )
            nv = st([SPC, 1], "nv")
            na = st([SPC, 1], "na")
            nv_i = nc.scalar.activation(out=nv[:], in_=ssv[:], func=AF.Sqrt)
            na_i = nc.scalar.activation(out=na[:], in_=ssa[:], func=AF.Sqrt)
            if sq_insts:
                mid_sq = sq_insts[-1]
                bass_rust.add_dep_helper(
                    nv_i.ins, mid_sq.ins, sync=True,
                    reason="ACT queue: anchor sqrts after squares")
                bass_rust.add_dep_helper(
                    na_i.ins, mid_sq.ins, sync=True,
                    reason="ACT queue: anchor sqrts after squares")
            nc.vector.tensor_scalar_max(out=nv[:], in0=nv[:], scalar1=EPS)
            nc.vector.tensor_scalar_max(out=na[:], in0=na[:], scalar1=EPS)
            rv = st([SPC, 1], "rv")
            ra = st([SPC, 1], "ra")
            nc.vector.reciprocal(out=rv[:], in_=nv[:])
            nc.vector.reciprocal(out=ra[:], in_=na[:])
            # per-stream exp scale: scal4[p, 2s+tn] = invnorm(other anchor)/TEMP
            rvec = st([SPC, 2], "rvec")
            nc.vector.tensor_scalar(
                out=rvec[:, 0:1], in0=ra[:], scalar1=1.0 / TEMP, scalar2=None,
                op0=ALU.mult)
            nc.vector.tensor_scalar(
                out=rvec[:, 1:2], in0=rv[:], scalar1=1.0 / TEMP, scalar2=None,
                op0=ALU.mult)
            scal4_ps = ps1.tile([P, NST], F32, tag="scal4_ps", name="scal4_ps")
            for s in range(SPC):
                nc.tensor.matmul(
                    out=scal4_ps[:, 2 * s : 2 * s + 2],
                    lhsT=sel[:, s * P : (s + 1) * P], rhs=rvec[:],
                    start=True, stop=True)
            scal4 = st([P, NST], "scal4")
            nc.vector.tensor_copy(out=scal4[:], in_=scal4_ps[:])
            # pos
            praw = st([SPC, 1], "praw")
            scr5 = st([SPC, D], "scr5")
            nc.vector.scalar_tensor_tensor(
                out=scr5[:], in0=anc_v[:], scalar=1.0, in1=anc_a[:],
                op0=ALU.mult, op1=ALU.mult, accum_out=praw[:],
            )
            q1 = st([SPC, 1], "q1")
            nc.vector.tensor_tensor(out=q1[:], in0=rv[:], in1=ra[:], op=ALU.mult)
            pos = st([SPC, 1], "pos")
            nc.vector.scalar_tensor_tensor(
                out=pos[:], in0=praw[:], scalar=1.0 / TEMP, in1=q1[:],
                op0=ALU.mult, op1=ALU.mult,
            )

            W = NST * TPS  # 128
            nall = st([P, W], "nall")
            rall = st([P, W], "rall")
            sim = st([P, W], "sim")
            sqrt_inst = nc.scalar.activation(
                out=nall[:], in_=ssq[:], func=AF.Sqrt)
            nc.vector.tensor_scalar_max(out=nall[:], in0=nall[:], scalar1=EPS)
            nc.vector.reciprocal(out=rall[:], in_=nall[:])
            nc.vector.tensor_tensor(
                out=sim[:], in0=dots[:], in1=rall[:], op=ALU.mult)
            ewarm = st([1, 1], "ewarm")
            ewarm_i = nc.scalar.activation(out=ewarm[:], in_=warm[:], func=AF.Ln)
            bass_rust.add_dep_helper(
                ewarm_i.ins, sqrt_inst.ins, sync=True,
                reason="exp/ln table load after last sqrt")
            eall = st([P, W], "eall")
            Scols = st([P, NST], "Scols")
            for i in range(NST):
                e_i = nc.scalar.activation(
                    out=eall[:, i * TPS : (i + 1) * TPS],
                    in_=sim[:, i * TPS : (i + 1) * TPS],
                    func=AF.Exp, scale=scal4[:, i : i + 1],
                    accum_out=Scols[:, i : i + 1],
                )
                bass_rust.add_dep_helper(
                    e_i.ins, sqrt_inst.ins, sync=True,
                    reason="all exps after last sqrt (one table switch)")
            SP = ps1.tile([1, NST], F32, tag="SP", name="SP")
            nc.tensor.matmul(
                out=SP[:], lhsT=ones[:], rhs=Scols[:], start=True, stop=True)
            pos_t = ps1.tile([1, SPC], F32, tag="pos_t", name="pos_t")
            nc.tensor.transpose(
                out=pos_t[:], in_=pos[:], identity=ident[:SPC, :SPC])
            epi = st([1, 8], "epi")
            nc.vector.tensor_copy(out=epi[:, 0:NST], in_=SP[:])
            nc.vector.tensor_copy(out=epi[:, 4 : 4 + SPC], in_=pos_t[:, 0:SPC])
            pexp_inst = nc.scalar.activation(
                out=epi[:, 6 : 6 + SPC], in_=epi[:, 4 : 4 + SPC], func=AF.Exp)
            bass_rust.add_dep_helper(
                pexp_inst.ins, sqrt_inst.ins, sync=True,
                reason="group ACT table sets: exp after sqrt")
            negs = st([1, NST], "negs")
            for s in range(SPC):
                nc.vector.tensor_scalar(
                    out=negs[:, 2 * s : 2 * s + 2],
                    in0=epi[:, 2 * s : 2 * s + 2],
                    scalar1=epi[:, 6 + s : 7 + s],
                    scalar2=None,
                    op0=ALU.subtract,
                )
            lns = st([1, NST], "lns")
            nc.scalar.activation(out=lns[:], in_=negs[:], func=AF.Ln)
            lsum = st([1, 1], "lsum")
            nc.vector.reduce_sum(out=lsum[:], in_=lns[:], axis=AX.X)
            psum2 = st([1, 1], "psum2")
            nc.vector.reduce_sum(
                out=psum2[:], in_=epi[:, 4 : 4 + SPC], axis=AX.X)
            res = st([1, 1], "res")
            nc.vector.scalar_tensor_tensor(
                out=res[:], in0=lsum[:], scalar=0.5, in1=psum2[:],
                op0=ALU.mult, op1=ALU.subtract,
            )
            nc.sync.dma_start(out=out[:], in_=res[:])

    nc.compile()
    return nc


_NC = None


def _get_nc():
    global _NC
    if _NC is None:
        _NC = _build()
    return _NC


def _to_bf16(x):
    import ml_dtypes
    return x.astype(ml_dtypes.bfloat16)


def make_in_maps(video, audio, mask):
    video = np.asarray(video, dtype=np.float32)
    audio = np.asarray(audio, dtype=np.float32)
    mask = np.asarray(mask, dtype=np.int32)
    in_maps = []
    for c in range(NCORES):
        sl = slice(c * SPC, (c + 1) * SPC)
        v = np.ascontiguousarray(video[sl])
        a = np.ascontiguousarray(audio[sl])
        in_maps.append({
            "vT": np.ascontiguousarray(_to_bf16(v).transpose(0, 2, 1)),
            "aT": np.ascontiguousarray(_to_bf16(a).transpose(0, 2, 1)),
            "v32": v,
            "a32": a,
            "mask": np.ascontiguousarray(mask[sl]),
        })
    return in_maps


def kernel(video, audio, mask):
    nc = _get_nc()
    in_maps = make_in_maps(video, audio, mask)
    res = run_bass_kernel_spmd(nc, in_maps, core_ids=list(range(NCORES)))
    total = np.float64(0.0)
    for r in res.results:
        total += np.float64(r["out"][0, 0])
    return np.array([total / B], dtype=np.float32)


# revision 15
# speedup vs baseline: 1.1660x; 1.0097x over previous
"""Trainium2 Bass kernel for nn_ContrastiveLoss (v3: transposed layout).

Full inputs: video [16, 4096, 256] f32, audio [16, 4096, 256] f32,
mask [16, 4096] int32. Output: scalar loss, shape (1,) f32.

Sharding: data-parallel over batch B=16 -> 2 samples per core on 8 cores.
Host sums the 8 partial losses and divides by B.

Per-core plan (all heavy data in bf16, TRANSPOSED host-side to [s, d, t]):
  - mask argmax (first-max tie-break) -> idx_s; indirect-DMA gather of the
    f32 anchor rows from a natural-layout f32 copy (read-only, 2 rows).
  - anchors transposed on PE to per-partition columns, cast to bf16; they
    become matmul WEIGHTS.
  - squares: sq = x*x elementwise on ACT (Square) / DVE (tt 2x) / GPSIMD,
    one [128, 2048] half-tile at a time, bf16.
  - PE reduces over d (the partition dim): dot rows via lhsT=anchor column,
    sumsq rows via lhsT=ones; [8, 512]-chunk PSUM tiles accumulate the two
    128-d chunks; a single DVE/ACT copy evacuates 4096 values at once.
  - results bounce through a DRAM scratch to land t-on-partitions
    ([128, 128] f32 natural tiles), then the scalar epilogue (same as the
    f32 baseline): sim = dots * rsqrt(ssq) * 1/(||anc||*TEMP); per-stream
    S = sum_t exp(sim); neg = S - exp(pos); loss partial from pos/negs.
"""

import numpy as np

import bass_rust
import concourse.bass as bass
import concourse.mybir as mybir
from concourse import bacc
from concourse.bass_utils import run_bass_kernel_spmd
from concourse.tile import TileContext

B, T, D = 16, 4096, 256
NCORES = 8
SPC = B // NCORES          # samples per core
TEMP = 0.07
EPS = 1e-8
P = 128
TPS = T // P               # 32 (natural epilogue tiling: t = p*TPS + j)
NST = 2 * SPC              # streams: (s, video/audio)
NCH = D // P               # 2 d-chunks
JC = 512                   # PSUM reduce chunk (t columns)
NJ = T // JC               # 8 j-chunks per stream
HALF = T // 2              # square half-tile

F32 = mybir.dt.float32
BF16 = mybir.dt.bfloat16
I32 = mybir.dt.int32
AF = mybir.ActivationFunctionType
ALU = mybir.AluOpType
AX = mybir.AxisListType

# static engine assignment for the 16 squaring half-tiles, keyed by
# (stream, chunk, half) order of arrival; balanced so DVE/ACT/GPSIMD all
# finish early and the last halves go to the fastest engines.
# streams arrive v0, a0, v1, a1; chunks c0, c1; halves h0, h1.
_SQ_ENGINE = {}
_order = [(q, c, h) for q in range(NST) for c in range(NCH) for h in range(2)]
_pattern = ["ACT", "DVE", "ACT", "GP", "ACT", "DVE", "ACT", "GP",
            "ACT", "DVE", "ACT", "DVE", "ACT", "DVE", "DVE", "DVE"]
for _k, _e in zip(_order, _pattern):
    _SQ_ENGINE[_k] = _e


def _build():
    nc = bacc.Bacc(trn_type="TRN2", debug=False)

    vT = nc.dram_tensor("vT", [SPC, D, T], BF16, kind="ExternalInput")
    aT = nc.dram_tensor("aT", [SPC, D, T], BF16, kind="ExternalInput")
    v32 = nc.dram_tensor("v32", [SPC, T, D], F32, kind="ExternalInput")
    a32 = nc.dram_tensor("a32", [SPC, T, D], F32, kind="ExternalInput")
    mask = nc.dram_tensor("mask", [SPC, T], I32, kind="ExternalInput")
    scr = nc.dram_tensor("scr", [2, NST, T], F32, kind="Internal")
    out = nc.dram_tensor("out", [1, 1], F32, kind="ExternalOutput")

    # constants
    jj = np.arange(TPS, dtype=np.float32)
    pp = np.arange(P, dtype=np.float32)[:, None]
    revt_np = 4095.0 - (pp * TPS + jj[None, :])         # [128, 32]
    revt_np = np.tile(revt_np, (1, SPC)).astype(np.float32)
    revt_d = nc.inline_tensor(revt_np, name="revt")
    ident_d = nc.inline_tensor(np.eye(P, dtype=np.float32), name="ident")
    ones_d = nc.inline_tensor(np.ones((P, 1), dtype=np.float32), name="ones")
    srow_d = nc.inline_tensor(
        (np.arange(SPC, dtype=np.int32) * T)[:, None], name="srow")
    sel_np = np.zeros((SPC, SPC * P), dtype=np.float32)
    for s in range(SPC):
        sel_np[s, s * P : (s + 1) * P] = 1.0
    sel_d = nc.inline_tensor(sel_np, name="sel")

    vflat = v32[:].rearrange("s t d -> (s t) d")
    aflat = a32[:].rearrange("s t d -> (s t) d")
    mtile = mask[:].rearrange("s (p j) -> p s j", p=P)   # [128, SPC, 32]
    scr4 = scr[:]                                        # [2, NST, T]

    with TileContext(nc) as tc:
        with (
            tc.tile_pool(name="sb", bufs=1) as sb,
            tc.tile_pool(name="data", bufs=6) as data,
            tc.tile_pool(name="sqp", bufs=5) as sqp,
            tc.tile_pool(name="ev", bufs=3) as evp,
            tc.tile_pool(name="ps", bufs=3, space="PSUM") as ps,
            tc.tile_pool(name="ps1", bufs=1, space="PSUM") as ps1,
        ):
            def st(shape, tag, dtype=F32, pool=sb):
                return pool.tile(shape, dtype, tag=tag, name=tag)

            # ---- constants to SBUF ----
            revt = st([P, SPC * TPS], "revt")
            ident = st([P, P], "ident")
            ones = st([P, 1], "ones")
            srow = st([SPC, 1], "srow", dtype=I32)
            sel = st([SPC, SPC * P], "sel")
            nc.sync.dma_start(out=revt[:], in_=revt_d[:])
            nc.sync.dma_start(out=ident[:], in_=ident_d[:])
            nc.sync.dma_start(out=ones[:], in_=ones_d[:])
            nc.sync.dma_start(out=srow[:], in_=srow_d[:])
            nc.sync.dma_start(out=sel[:], in_=sel_d[:])
            ones16 = st([P, 1], "ones16", dtype=BF16)
            nc.vector.tensor_copy(out=ones16[:], in_=ones[:])

            # mask first: the argmax->anchor chain gates the dot matmuls
            mf = st([P, SPC * TPS], "mf")
            mf3 = mf[:].rearrange("p (s j) -> p s j", s=SPC)
            nc.gpsimd.dma_start(out=mf3, in_=mtile)  # int32 -> f32 cast

            # PE p-state warmup: dummy matmul spin during the DMA prologue
            # so the reduce matmuls run at full clock.
            dumm = st([P, JC], "dumm", dtype=BF16)
            nc.gpsimd.memset(dumm[:], 1.0)
            wps = ps.tile([8, JC], F32, tag="red", name="warm_ps")
            for _w in range(20):
                nc.tensor.matmul(
                    out=wps[:], lhsT=dumm[:, 0:8], rhs=dumm[:],
                    start=(_w == 0), stop=(_w == 19),
                )

            # ---- bulk loads: per (stream, chunk) [128, T] bf16, 2 halves
            bigs = {}
            srcT = {0: vT, 1: aT}
            for q in range(NST):
                s, tn = divmod(q, 2)
                for c in range(NCH):
                    big = data.tile([P, T], BF16, tag="big", name="big")
                    for h in range(2):
                        nc.sync.dma_start(
                            out=big[:, h * HALF : (h + 1) * HALF],
                            in_=srcT[tn][s, c * P : (c + 1) * P,
                                         h * HALF : (h + 1) * HALF],
                        )
                    bigs[(q, c)] = big

            # warm the ACT sqrt/square table set off the critical path
            warm = st([1, 1], "warm")
            nc.scalar.activation(out=warm[:], in_=ones[0:1, :], func=AF.Sqrt)

            # ---- Stage A: argmax of mask per sample ----
            comb = st([P, SPC * TPS], "comb")
            nc.vector.scalar_tensor_tensor(
                out=comb[:], in0=mf[:], scalar=4096.0, in1=revt[:],
                op0=ALU.mult, op1=ALU.add,
            )
            pmax = st([P, SPC], "pmax")
            for s in range(SPC):
                nc.vector.reduce_max(
                    out=pmax[:, s : s + 1],
                    in_=comb[:, s * TPS : (s + 1) * TPS],
                    axis=AX.X,
                )
            pmax_t = ps1.tile([SPC, P], F32, tag="pmax_t", name="pmax_t")
            nc.tensor.transpose(out=pmax_t[:], in_=pmax[:], identity=ident[:])
            Mf = st([SPC, 1], "Mf")
            nc.vector.reduce_max(out=Mf[:], in_=pmax_t[:], axis=AX.X)
            Mi = st([SPC, 1], "Mi", dtype=I32)
            nc.vector.tensor_copy(out=Mi[:], in_=Mf[:])
            ridx = st([SPC, 1], "ridx", dtype=I32)
            nc.vector.tensor_scalar(
                out=ridx[:], in0=Mi[:], scalar1=4095, scalar2=None,
                op0=ALU.bitwise_and,
            )
            idx = st([SPC, 1], "idx", dtype=I32)
            nc.vector.tensor_scalar(
                out=idx[:], in0=ridx[:], scalar1=-1, scalar2=4095,
                op0=ALU.mult, op1=ALU.add,
            )
            gidx = st([SPC, 1], "gidx", dtype=I32)
            nc.vector.tensor_tensor(
                out=gidx[:], in0=idx[:], in1=srow[:], op=ALU.add)

            # ---- Stage B: gather f32 anchors ----
            anc_v = st([SPC, D], "anc_v")
            anc_a = st([SPC, D], "anc_a")
            nc.gpsimd.indirect_dma_start(
                out=anc_v[:], out_offset=None, in_=vflat,
                in_offset=bass.IndirectOffsetOnAxis(ap=gidx[:, :1], axis=0),
            )
            nc.gpsimd.indirect_dma_start(
                out=anc_a[:], out_offset=None, in_=aflat,
                in_offset=bass.IndirectOffsetOnAxis(ap=gidx[:, :1], axis=0),
            )

            # ---- Stage C: transpose anchors to per-partition columns ----
            # ancT16 cols: c * NST + tensor * SPC + s  (tensor 0=v, 1=a)
            ancT16 = st([P, NCH * NST], "ancT16", dtype=BF16)
            for c in range(NCH):
                for tn, anc in ((0, anc_v), (1, anc_a)):
                    atp = ps1.tile([P, SPC], F32, tag="atp", name="atp")
                    nc.tensor.transpose(
                        out=atp[:], in_=anc[:, c * P : (c + 1) * P],
                        identity=ident[:SPC, :SPC],
                    )
                    o = c * NST + tn * SPC
                    nc.vector.tensor_copy(
                        out=ancT16[:, o : o + SPC], in_=atp[:])

            # dot-weight column for stream q chunk c: the OTHER tensor's
            # anchor of the same sample.
            def dotw(q, c):
                s, tn = divmod(q, 2)
                col = c * NST + (1 - tn) * SPC + s
                return ancT16[:, col : col + 1]

            # PE out base partition must be 0: each reduce matmul writes the
            # full [8, JC] tile through a [128, 8] weight mat that is zero
            # except its target column. Static ones-mats for sq rows
            # (col 2jj+1), runtime anchor-mats for dot rows (col 2jj).
            onesm_np = np.zeros((P, 4 * 8), dtype=np.float32)
            for _jj in range(4):
                onesm_np[:, _jj * 8 + 2 * _jj + 1] = 1.0
            onesm_d = nc.inline_tensor(onesm_np, name="onesm")
            onesm_f = st([P, 32], "onesm_f")
            nc.sync.dma_start(out=onesm_f[:], in_=onesm_d[:])
            onesm = st([P, 32], "onesm", dtype=BF16)
            nc.vector.tensor_copy(out=onesm[:], in_=onesm_f[:])

            wqs = {}
            for q in range(NST):
                for c in range(NCH):
                    wq = st([P, 32], f"wq{q}{c}", dtype=BF16)
                    nc.gpsimd.memset(wq[:], 0.0)
                    for jj in range(4):
                        nc.vector.tensor_copy(
                            out=wq[:, jj * 8 + 2 * jj : jj * 8 + 2 * jj + 1],
                            in_=dotw(q, c),
                        )
                    wqs[(q, c)] = wq

            # ---- Stage D: squares + PE reduces + evac/bounce ----
            sq_insts = []
            sqs = {}
            for q in range(NST):
                for c in range(NCH):
                    big = bigs[(q, c)]
                    sq = sqp.tile([P, T], BF16, tag="sq", name="sq")
                    for h in range(2):
                        eng = _SQ_ENGINE[(q, c, h)]
                        sl = slice(h * HALF, (h + 1) * HALF)
                        if eng == "ACT":
                            i = nc.scalar.activation(
                                out=sq[:, sl], in_=big[:, sl], func=AF.Square)
                            sq_insts.append(i)
                        elif eng == "DVE":
                            nc.vector.tensor_tensor(
                                out=sq[:, sl], in0=big[:, sl], in1=big[:, sl],
                                op=ALU.mult)
                        else:
                            nc.gpsimd.tensor_tensor(
                                out=sq[:, sl], in0=big[:, sl], in1=big[:, sl],
                                op=ALU.mult)
                    sqs[(q, c)] = sq

            # PE reduce per stream, j-group of 4: PSUM [8, 512]
            # rows: (j_in_group * 2) + {0: dot, 1: sq}
            evac_dmas = []
            for q in range(NST):
                for g in range(2):
                    pt = ps.tile([8, JC], F32, tag="red", name="red")
                    nmm = 4 * NCH * 2
                    k = 0
                    for jj in range(4):
                        j = g * 4 + jj
                        sl = slice(j * JC, (j + 1) * JC)
                        for c in range(NCH):
                            nc.tensor.matmul(
                                out=pt[:],
                                lhsT=wqs[(q, c)][:, jj * 8 : (jj + 1) * 8],
                                rhs=bigs[(q, c)][:, sl],
                                start=(k == 0), stop=(k == nmm - 1),
                            )
                            k += 1
                            nc.tensor.matmul(
                                out=pt[:],
                                lhsT=onesm[:, jj * 8 : (jj + 1) * 8],
                                rhs=sqs[(q, c)][:, sl],
                                start=(k == 0), stop=(k == nmm - 1),
                            )
                            k += 1
                    ev = evp.tile([8, JC], F32, tag="ev", name="ev")
                    if q % 2 == 0:
                        nc.vector.tensor_copy(out=ev[:], in_=pt[:])
                    else:
                        nc.scalar.activation(
                            out=ev[:], in_=pt[:], func=AF.Copy)
                    # rows (j, qty) -> scr[qty, q, j*JC : ...]
                    ed = nc.sync.dma_start(
                        out=scr4[:, q, g * 4 * JC : (g + 1) * 4 * JC]
                        .rearrange("qty (j cc) -> j qty cc", cc=JC),
                        in_=ev[:],
                    )
                    evac_dmas.append(ed)

            # ---- reload natural [128, NST*TPS] ----
            dots = st([P, NST * TPS], "dots")
            ssq = st([P, NST * TPS], "ssq")
            # col = q*TPS + j ; t = p*TPS + j
            rl1 = nc.sync.dma_start(
                out=dots[:].rearrange("p (q j) -> p q j", j=TPS),
                in_=scr4[0].rearrange("q (p j) -> p q j", j=TPS),
            )
            rl2 = nc.sync.dma_start(
                out=ssq[:].rearrange("p (q j) -> p q j", j=TPS),
                in_=scr4[1].rearrange("q (p j) -> p q j", j=TPS),
            )
            # Tile does not track DRAM read-after-write: order the natural
            # reloads after every scratch evacuation explicitly.
            for ed in evac_dmas:
                bass_rust.add_dep_helper(
                    rl1.ins, ed.ins, sync=True, reason="scr RAW")
                bass_rust.add_dep_helper(
                    rl2.ins, ed.ins, sync=True, reason="scr RAW")

            # ---- Stage E: epilogue (same math as baseline) ----
            scr2 = st([SPC, D], "scr2")
            scr3 = st([SPC, D], "scr3")
            ssv = st([SPC, 1], "ssv")
            ssa = st([SPC, 1], "ssa")
            nc.vector.scalar_tensor_tensor(
                out=scr2[:], in0=anc_v[:], scalar=1.0, in1=anc_v[:],
                op0=ALU.mult, op1=ALU.mult, accum_out=ssv[:],
            )
            nc.vector.scalar_tensor_tensor(
                out=scr3[:], in0=anc_a[:], scalar=1.0, in1=anc_a[:],
                op0=ALU.mult, op1=ALU.mult, accum_out=ssa[:],
            )
            nv = st([SPC, 1], "nv")
            na = st([SPC, 1], "na")
            nv_i = nc.scalar.activation(out=nv[:], in_=ssv[:], func=AF.Sqrt)
            na_i = nc.scalar.activation(out=na[:], in_=ssa[:], func=AF.Sqrt)
            if sq_insts:
                mid_sq = sq_insts[-1]
                bass_rust.add_dep_helper(
                    nv_i.ins, mid_sq.ins, sync=True,
                    reason="ACT queue: anchor sqrts after squares")
                bass_rust.add_dep_helper(
                    na_i.ins, mid_sq.ins, sync=True,
                    reason="ACT queue: anchor sqrts after squares")
            nc.vector.tensor_scalar_max(out=nv[:], in0=nv[:], scalar1=EPS)
            nc.vector.tensor_scalar_max(out=na[:], in0=na[:], scalar1=EPS)
            rv = st([SPC, 1], "rv")
            ra = st([SPC, 1], "ra")
            nc.vector.reciprocal(out=rv[:], in_=nv[:])
            nc.vector.reciprocal(out=ra[:], in_=na[:])
            # per-stream exp scale: scal4[p, 2s+tn] = invnorm(other anchor)/TEMP
            rvec = st([SPC, 2], "rvec")
            nc.vector.tensor_scalar(
                out=rvec[:, 0:1], in0=ra[:], scalar1=1.0 / TEMP, scalar2=None,
                op0=ALU.mult)
            nc.vector.tensor_scalar(
                out=rvec[:, 1:2], in0=rv[:], scalar1=1.0 / TEMP, scalar2=None,
                op0=ALU.mult)
            scal4_ps = ps1.tile([P, NST], F32, tag="scal4_ps", name="scal4_ps")
            for s in range(SPC):
                nc.tensor.matmul(
                    out=scal4_ps[:, 2 * s : 2 * s + 2],
                    lhsT=sel[:, s * P : (s + 1) * P], rhs=rvec[:],
                    start=True, stop=True)
            scal4 = st([P, NST], "scal4")
            nc.vector.tensor_copy(out=scal4[:], in_=scal4_ps[:])
            # pos
            praw = st([SPC, 1], "praw")
            scr5 = st([SPC, D], "scr5")
            nc.vector.scalar_tensor_tensor(
                out=scr5[:], in0=anc_v[:], scalar=1.0, in1=anc_a[:],
                op0=ALU.mult, op1=ALU.mult, accum_out=praw[:],
            )
            q1 = st([SPC, 1], "q1")
            nc.vector.tensor_tensor(out=q1[:], in0=rv[:], in1=ra[:], op=ALU.mult)
            pos = st([SPC, 1], "pos")
            nc.vector.scalar_tensor_tensor(
                out=pos[:], in0=praw[:], scalar=1.0 / TEMP, in1=q1[:],
                op0=ALU.mult, op1=ALU.mult,
            )

            W = NST * TPS  # 128
            nall = st([P, W], "nall")
            rall = st([P, W], "rall")
            sim = st([P, W], "sim")
            sqrt_inst = nc.scalar.activation(
                out=nall[:], in_=ssq[:], func=AF.Sqrt)
            nc.vector.tensor_scalar_max(out=nall[:], in0=nall[:], scalar1=EPS)
            nc.vector.reciprocal(out=rall[:], in_=nall[:])
            nc.vector.tensor_tensor(
                out=sim[:], in0=dots[:], in1=rall[:], op=ALU.mult)
            ewarm = st([1, 1], "ewarm")
            ewarm_i = nc.scalar.activation(out=ewarm[:], in_=warm[:], func=AF.Ln)
            bass_rust.add_dep_helper(
                ewarm_i.ins, sqrt_inst.ins, sync=True,
                reason="exp/ln table load after last sqrt")
            eall = st([P, W], "eall")
            Scols = st([P, NST], "Scols")
            for i in range(NST):
                e_i = nc.scalar.activation(
                    out=eall[:, i * TPS : (i + 1) * TPS],
                    in_=sim[:, i * TPS : (i + 1) * TPS],
                    func=AF.Exp, scale=scal4[:, i : i + 1],
                    accum_out=Scols[:, i : i + 1],
                )
                bass_rust.add_dep_helper(
                    e_i.ins, sqrt_inst.ins, sync=True,
                    reason="all exps after last sqrt (one table switch)")
            SP = ps1.tile([1, NST], F32, tag="SP", name="SP")
            nc.tensor.matmul(
                out=SP[:], lhsT=ones[:], rhs=Scols[:], start=True, stop=True)
            pos_t = ps1.tile([1, SPC], F32, tag="pos_t", name="pos_t")
            nc.tensor.transpose(
                out=pos_t[:], in_=pos[:], identity=ident[:SPC, :SPC])
            epi = st([1, 8], "epi")
            nc.vector.tensor_copy(out=epi[:, 0:NST], in_=SP[:])
            nc.vector.tensor_copy(out=epi[:, 4 : 4 + SPC], in_=pos_t[:, 0:SPC])
            pexp_inst = nc.scalar.activation(
                out=epi[:, 6 : 6 + SPC], in_=epi[:, 4 : 4 + SPC], func=AF.Exp)
            bass_rust.add_dep_helper(
                pexp_inst.ins, sqrt_inst.ins, sync=True,
                reason="group ACT table sets: exp after sqrt")
            negs = st([1, NST], "negs")
            for s in range(SPC):
                nc.vector.tensor_scalar(
                    out=negs[:, 2 * s : 2 * s + 2],
                    in0=epi[:, 2 * s : 2 * s + 2],
                    scalar1=epi[:, 6 + s : 7 + s],
                    scalar2=None,
                    op0=ALU.subtract,
                )
            lns = st([1, NST], "lns")
            nc.scalar.activation(out=lns[:], in_=negs[:], func=AF.Ln)
            lsum = st([1, 1], "lsum")
            nc.vector.reduce_sum(out=lsum[:], in_=lns[:], axis=AX.X)
            psum2 = st([1, 1], "psum2")
            nc.vector.reduce_sum(
                out=psum2[:], in_=epi[:, 4 : 4 + SPC], axis=AX.X)
            res = st([1, 1], "res")
            nc.vector.scalar_tensor_tensor(
                out=res[:], in0=lsum[:], scalar=0.5, in1=psum2[:],
                op0=ALU.mult, op1=ALU.subtract,
            )
            nc.sync.dma_start(out=out[:], in_=res[:])

    nc.compile()
    return nc


_NC = None


def _get_nc():
    global _NC
    if _NC is None:
        _NC = _build()
    return _NC


def _to_bf16(x):
    import ml_dtypes
    return x.astype(ml_dtypes.bfloat16)


def make_in_maps(video, audio, mask):
    video = np.asarray(video, dtype=np.float32)
    audio = np.asarray(audio, dtype=np.float32)
    mask = np.asarray(mask, dtype=np.int32)
    in_maps = []
    for c in range(NCORES):
        sl = slice(c * SPC, (c + 1) * SPC)
        v = np.ascontiguousarray(video[sl])
        a = np.ascontiguousarray(audio[sl])
        in_maps.append({
            "vT": np.ascontiguousarray(_to_bf16(v).transpose(0, 2, 1)),
            "aT": np.ascontiguousarray(_to_bf16(a).transpose(0, 2, 1)),
            "v32": v,
            "a32": a,
            "mask": np.ascontiguousarray(mask[sl]),
        })
    return in_maps


def kernel(video, audio, mask):
    nc = _get_nc()
    in_maps = make_in_maps(video, audio, mask)
    res = run_bass_kernel_spmd(nc, in_maps, core_ids=list(range(NCORES)))
    total = np.float64(0.0)
    for r in res.results:
        total += np.float64(r["out"][0, 0])
    return np.array([total / B], dtype=np.float32)
